# revision 16
# baseline (speedup 1.0000x reference)
"""Trainium2 Bass kernel for Tacotron-style location-sensitive attention.

Reference computation (per batch row b):
    key   = memory @ Wm.T + bm                       # [T, A]
    q     = query @ Wq.T + bq                        # [A]
    loc   = (conv1d(awc, conv_w) + conv_b)' @ Wloc'  # [T, A]
    e     = tanh(q + loc + key) @ Wv.T + bv          # [T]
    w     = softmax(where(mask, -inf, e))            # [T]
    ctx   = w @ memory                               # [M]
Outputs: (attention_context [B, M], attention_weights [B, T])

Sharding: pure data parallel, batch B=64 split over 8 NeuronCores (8 rows
per core).  All weights are replicated.

Per-core device program:
  * memory is DMA'd in its natural [t, m] layout as [125, 8*512] tiles and
    transposed on the PE (identity matmuls) into memT [m=128, 4*1000] for
    the key projection; the natural tiles are reused by the context matmul.
  * energies accumulate in PSUM in [a=128, t] orientation; the location
    conv is folded into the same accumulation group as a K=31 matmul
    against a sliding-window access pattern on a zero-padded copy of awc
    (CW = Wloc @ conv_w precomputed on host).
  * q + bq + bm + Wloc@conv_b enters as the per-partition bias of the tanh.
  * the mask enters as a K=1 matmul adding mask*-1e30 to the energies after
    the Wv reduction (softmax then gives exactly 0 for masked slots).
  * softmax skips max-subtraction (energies are O(1) so exp cannot
    overflow); exp produces unnormalized u with Z from the activation's
    accum_out; the context matmul consumes u and is scaled by 1/Z after.
  * the context matmul for batch b is emitted one iteration late so the
    PE stays busy with batch b+1's transposes while u_b round-trips
    through DRAM (to reshape [1,1000] -> [125,8] stationary columns).
  * attention_weights = u/Z for all rows in one batched pass at the end.
"""

import numpy as np
from contextlib import ExitStack

B, T = 64, 1000
Q_DIM, M_DIM, A_DIM = 1024, 512, 128
N_FILT, KSIZE = 32, 31
PAD = (KSIZE - 1) // 2
N_CORES = 8
BL = B // N_CORES            # 8 batch rows per core
# t-chunk sizes: all even (fp32r matmul ISA requires even innermost
# element counts) and summing to T: 4*126 + 4*124 = 1000
CH = [126, 126, 126, 126, 124, 124, 124, 124]
CHOFF = [sum(CH[:i]) for i in range(len(CH))]
TCMAX = CH[0]                # 126
NTC = len(CH)               # 8 t-chunks
NMC = M_DIM // 128           # 4 m-chunks
NQC = Q_DIM // 128           # 8 q-chunks
TPAD = T + 2 * PAD           # 1030

_PROG_CACHE = {}


def _emit(ctx, tc_, dram):
    import concourse.bass as bass
    from concourse import mybir

    nc = tc_.nc
    f32 = mybir.dt.float32
    f32r = mybir.dt.float32r
    AF = mybir.ActivationFunctionType

    def r(ap):
        return ap

    mem_d = dram["memory"]
    query_d = dram["query"]
    awc_d = dram["awc"]
    maskneg_d = dram["maskneg"]
    wm_d = dram["wm_pack"]
    wq_d = dram["wq_pack"]
    cw_d = dram["cw_pack"]
    wv_d = dram["wv_pack"]
    biasc_d = dram["bias_c"]
    ident_d = dram["ident"]
    ones_d = dram["ones1"]
    out_ctx_d = dram["out_ctx"]
    out_w_d = dram["out_w"]
    awcpad_d = dram["awcpad_scr"]
    uscr_d = dram["u_scr"]
    rzscr_d = dram["rz_scr"]

    # ---------------- pools ----------------
    wpool = ctx.enter_context(tc_.tile_pool(name="weights", bufs=1))
    setup = ctx.enter_context(tc_.tile_pool(name="setup", bufs=1))
    mempool = ctx.enter_context(tc_.tile_pool(name="mem", bufs=4))
    mtpool = ctx.enter_context(tc_.tile_pool(name="memT", bufs=2))
    tanhpool = ctx.enter_context(tc_.tile_pool(name="tanh", bufs=2))
    rowpool = ctx.enter_context(tc_.tile_pool(name="rows", bufs=2))
    rzpool = ctx.enter_context(tc_.tile_pool(name="rz", bufs=3))
    ucpool = ctx.enter_context(tc_.tile_pool(name="ucols", bufs=2))
    fpool = ctx.enter_context(tc_.tile_pool(name="final", bufs=1))
    upool = ctx.enter_context(tc_.tile_pool(name="udram", bufs=BL, space="DRAM"))

    pe_q = ctx.enter_context(tc_.tile_pool(name="ps_q", bufs=1, space="PSUM"))
    pe_tr = ctx.enter_context(tc_.tile_pool(name="ps_tr", bufs=2, space="PSUM"))
    pe_e = ctx.enter_context(tc_.tile_pool(name="ps_e", bufs=1, space="PSUM"))
    pe_en = ctx.enter_context(tc_.tile_pool(name="ps_en", bufs=1, space="PSUM"))
    pe_ctx = ctx.enter_context(tc_.tile_pool(name="ps_ctx", bufs=1, space="PSUM"))

    # ---------------- memory prefetch (first on the SWDGE ring) ----------
    mem_tiles = {}

    def load_mem(b):
        # natural layout: mem_sb[p, c*512 + m] = mem[b, CHOFF[c]+p, m]
        mem_sb = mempool.tile([TCMAX, NTC * M_DIM], f32r,
                              name=f"mem_sb{b}", tag="mem_sb")
        nc.gpsimd.dma_start(
            mem_sb[0:126, 0:4 * M_DIM].rearrange("p (c m) -> p c m", c=4),
            bass.AP(mem_d, b * T * M_DIM,
                    [[M_DIM, 126], [126 * M_DIM, 4], [1, M_DIM]]))
        nc.gpsimd.dma_start(
            mem_sb[0:124, 4 * M_DIM:8 * M_DIM].rearrange("p (c m) -> p c m", c=4),
            bass.AP(mem_d, (b * T + 504) * M_DIM,
                    [[M_DIM, 124], [124 * M_DIM, 4], [1, M_DIM]]))
        mem_tiles[b] = mem_sb

    PREFETCH = 3
    for b in range(min(PREFETCH, BL)):
        load_mem(b)

    # ---------------- weights + setup ----------------
    wm_sb = wpool.tile([128, NMC * 128], f32r)
    nc.sync.dma_start(wm_sb[:], wm_d.ap())
    wq_sb = wpool.tile([128, NQC * 128], f32r)
    nc.sync.dma_start(wq_sb[:], wq_d.ap())
    cw_sb = wpool.tile([KSIZE, 128], f32r)
    nc.sync.dma_start(cw_sb[:], cw_d.ap())
    wv_sb = wpool.tile([128, 1], f32r)
    nc.sync.dma_start(wv_sb[:], wv_d.ap())
    biasc_sb = wpool.tile([128, 1], f32)
    nc.sync.dma_start(biasc_sb[:], biasc_d.ap())
    ident_sb = wpool.tile([128, 128], f32r)
    nc.sync.dma_start(ident_sb[:], ident_d.ap())
    ones_sb = wpool.tile([1, 1], f32r)
    nc.sync.dma_start(ones_sb[:], ones_d.ap())
    # mask*-1e30 for all local rows, on one partition: [1, b*T + t]
    maskneg_sb = wpool.tile([1, BL * T], f32r)
    nc.sync.dma_start(maskneg_sb[:], bass.AP(maskneg_d, 0, [[1, 1], [1, BL * T]]))

    # zero-padded awc in DRAM: awcpad[b, :] = [0*15, awc[b, :], 0*15]
    padt = setup.tile([BL, TPAD], f32r)
    nc.vector.memset(padt[:].bitcast(mybir.dt.uint32), 0)
    nc.sync.dma_start(padt[:, PAD:PAD + T], awc_d.ap())
    nc.sync.dma_start(awcpad_d.ap(), padt[:])

    # sliding-window view of awcpad: win[k, b*T + t] = awcpad[b, t + k]
    win_sb = setup.tile([KSIZE, BL * T], f32r)
    win_src = bass.AP(awcpad_d, 0, [[1, KSIZE], [TPAD, BL], [1, T]])
    nc.sync.dma_start(win_sb[:].rearrange("k (b t) -> k b t", b=BL), win_src)

    # host-packed transposed query: qT[p, c*BL + b] = query[b, c*128 + p]
    qT_sb = setup.tile([128, NQC * BL], f32r)
    nc.sync.dma_start(qT_sb[:], query_d.ap())

    # q projection: psq[a, b] = sum_c WqT_c.T @ qT_c
    psq = pe_q.tile([128, BL], f32)
    for c in range(NQC):
        nc.tensor.matmul(
            psq[:], r(wq_sb[:, c * 128:(c + 1) * 128]),
            r(qT_sb[:, c * BL:(c + 1) * BL]),
            start=(c == 0), stop=(c == NQC - 1))
    # bias_q[a, b] = q[a, b] + (bq + bm + Wloc@conv_b)[a]
    bias_q = setup.tile([128, BL], f32)
    nc.scalar.activation(bias_q[:], psq[:], AF.Identity, bias=biasc_sb[:, 0:1])

    # ---------------- main per-batch loop ----------------
    # context matmul for batch b is emitted during iteration b+1 so the PE
    # fills the u_b DRAM round-trip latency with batch b+1's transposes.
    pend = None        # (mem_sb, ucols, rz, b) awaiting their context matmul
    u_drams = []

    def emit_ctx(p):
        p_mem, p_uc, p_rz, p_b = p
        psc = pe_ctx.tile([1, M_DIM], f32)
        for c in range(NTC):
            nc.tensor.matmul(psc[:], r(p_uc[0:CH[c], c:c + 1]),
                             r(p_mem[0:CH[c], c * M_DIM:(c + 1) * M_DIM]),
                             start=(c == 0), stop=(c == NTC - 1))
        ctx_row = rowpool.tile([1, M_DIM], f32)
        nc.scalar.activation(ctx_row[:], psc[0:1, :], AF.Copy,
                             scale=p_rz[0:1, 0:1])
        nc.sync.dma_start(bass.AP(out_ctx_d, p_b * M_DIM, [[1, 1], [1, M_DIM]]),
                          ctx_row[:])

    for b in range(BL):
        mem_sb = mem_tiles.pop(b)
        if b + PREFETCH < BL:
            load_mem(b + PREFETCH)

        # PE transpose -> memT[mp, mc*1000 + t] = mem[b, t, mc*128 + mp]
        memT = mtpool.tile([128, NMC * T], f32r)
        for tck in range(NTC):
            sz, off = CH[tck], CHOFF[tck]
            pst = pe_tr.tile([128, NMC * TCMAX], f32r, tag="pst")
            for mc in range(NMC):
                nc.tensor.transpose(
                    r(pst[:, mc * TCMAX:mc * TCMAX + sz]),
                    r(mem_sb[0:sz, tck * M_DIM + mc * 128: tck * M_DIM + (mc + 1) * 128]),
                    r(ident_sb[0:sz, 0:sz]))
            dst = memT[:].rearrange("p (mc t) -> p mc t", mc=NMC)[:, :, off:off + sz]
            src = pst[:].rearrange("p (mc t) -> p mc t", t=TCMAX)[:, :, 0:sz]
            if tck in (2, 6):
                nc.scalar.copy(dst, src)
            else:
                nc.vector.tensor_copy(dst, src)

        # energies accumulation: key (4 m-chunks) + location conv
        pse = pe_e.tile([128, T], f32)
        for t0, tw in ((0, 512), (512, 488)):
            for mc in range(NMC):
                nc.tensor.matmul(
                    pse[:, t0:t0 + tw],
                    r(wm_sb[:, mc * 128:(mc + 1) * 128]),
                    r(memT[:, mc * T + t0: mc * T + t0 + tw]),
                    start=(mc == 0), stop=False)
            nc.tensor.matmul(
                pse[:, t0:t0 + tw], r(cw_sb[:]),
                r(win_sb[:, b * T + t0: b * T + t0 + tw]),
                start=False, stop=True)

        # tanh(key + loc + (q + biases))
        tanh_sb = tanhpool.tile([128, T], f32r)
        nc.scalar.activation(tanh_sb[:], pse[:], AF.Tanh, bias=bias_q[:, b:b + 1])

        # energies: e[t] = Wv . tanh[:, t]  (+ mask * -1e30)
        psen = pe_en.tile([1, T], f32)
        for t0, tw in ((0, 512), (512, 488)):
            nc.tensor.matmul(psen[:, t0:t0 + tw], r(wv_sb[:]),
                             r(tanh_sb[:, t0:t0 + tw]), start=True, stop=False)
            nc.tensor.matmul(psen[:, t0:t0 + tw], r(ones_sb[:]),
                             r(maskneg_sb[:, b * T + t0: b * T + t0 + tw]),
                             start=False, stop=True)

        # u = exp(e) (no max subtraction: |e| is O(1)); Z via accum_out
        u_row = rowpool.tile([1, T], f32r)
        Z_t = rowpool.tile([1, 1], f32)
        nc.scalar.activation(u_row[:], psen[0:1, :], AF.Exp,
                             accum_out=Z_t[0:1, 0:1])
        rz_t = rzpool.tile([1, 1], f32)
        nc.vector.reciprocal(rz_t[:], Z_t[:])
        nc.sync.dma_start(bass.AP(rzscr_d, b, [[1, 1], [1, 1]]), rz_t[:])

        # u round-trip through DRAM to get [t-chunk, 1] stationary columns
        u_dram = upool.tile([1, T], f32r, name=f"u_dram{b}", tag="u_dram")
        nc.sync.dma_start(u_dram[:], u_row[:])
        ucols = ucpool.tile([TCMAX, NTC], f32r)
        nc.sync.dma_start(ucols[0:126, 0:4],
                          bass.AP(u_dram[:].tensor, 0, [[1, 126], [126, 4]]))
        nc.sync.dma_start(ucols[0:124, 4:8],
                          bass.AP(u_dram[:].tensor, 504, [[1, 124], [124, 4]]))

        if pend is not None:
            emit_ctx(pend)
        pend = (mem_sb, ucols, rz_t, b)
        u_drams.append(u_dram)

    emit_ctx(pend)

    # ---------------- final: w = u / Z for all rows ----------------
    rz_col = fpool.tile([BL, 1], f32)
    nc.sync.dma_start(rz_col[:], bass.AP(rzscr_d, 0, [[1, BL], [1, 1]]))
    u_all = fpool.tile([BL, T], f32)
    for b in range(BL):
        nc.gpsimd.dma_start(u_all[b:b + 1, :], u_drams[b][:])
    w_all = fpool.tile([BL, T], f32)
    nc.vector.tensor_scalar_mul(w_all[:], u_all[:], rz_col[:, 0:1])
    nc.sync.dma_start(out_w_d.ap(), w_all[:])


def _build():
    import concourse.bass as bass  # noqa: F401
    import concourse.tile as tile
    from concourse import bacc, mybir

    f32 = mybir.dt.float32
    f32r = mybir.dt.float32r

    nc = bacc.Bacc("TRN2", target_bir_lowering=False, debug=False)
    dram = {}
    dram["query"] = nc.dram_tensor("query", [128, NQC * BL], f32r, kind="ExternalInput")
    dram["memory"] = nc.dram_tensor("memory", [BL, T, M_DIM], f32r, kind="ExternalInput")
    dram["awc"] = nc.dram_tensor("awc", [BL, T], f32r, kind="ExternalInput")
    dram["maskneg"] = nc.dram_tensor("maskneg", [BL, T], f32r, kind="ExternalInput")
    dram["wm_pack"] = nc.dram_tensor("wm_pack", [128, NMC * 128], f32r, kind="ExternalInput")
    dram["wq_pack"] = nc.dram_tensor("wq_pack", [128, NQC * 128], f32r, kind="ExternalInput")
    dram["cw_pack"] = nc.dram_tensor("cw_pack", [KSIZE, 128], f32r, kind="ExternalInput")
    dram["wv_pack"] = nc.dram_tensor("wv_pack", [128, 1], f32r, kind="ExternalInput")
    dram["bias_c"] = nc.dram_tensor("bias_c", [128, 1], f32, kind="ExternalInput")
    dram["ident"] = nc.dram_tensor("ident", [128, 128], f32r, kind="ExternalInput")
    dram["ones1"] = nc.dram_tensor("ones1", [1, 1], f32r, kind="ExternalInput")
    dram["out_ctx"] = nc.dram_tensor("out_ctx", [BL, M_DIM], f32, kind="ExternalOutput")
    dram["out_w"] = nc.dram_tensor("out_w", [BL, T], f32, kind="ExternalOutput")
    dram["awcpad_scr"] = nc.dram_tensor("awcpad_scr", [BL, TPAD], f32r)
    dram["u_scr"] = nc.dram_tensor("u_scr", [BL, 1024], f32)
    dram["rz_scr"] = nc.dram_tensor("rz_scr", [1, BL], f32)

    with tile.TileContext(nc) as tc_:
        with ExitStack() as ctx:
            _emit(ctx, tc_, dram)
    nc.compile()
    return nc


def get_program():
    if "nc" not in _PROG_CACHE:
        _PROG_CACHE["nc"] = _build()
    return _PROG_CACHE["nc"]


def _host_pack(Wq, bq, Wm, bm, Wv, bv, conv_w, conv_b, Wloc):
    f32 = np.float32
    wm_pack = np.ascontiguousarray(
        Wm.T.reshape(NMC, 128, 128).transpose(1, 0, 2).reshape(128, NMC * 128), f32)
    wq_pack = np.ascontiguousarray(
        Wq.T.reshape(NQC, 128, 128).transpose(1, 0, 2).reshape(128, NQC * 128), f32)
    CW = Wloc @ conv_w[:, 0, :]                      # [128, 31]
    cw_pack = np.ascontiguousarray(CW.T, f32)        # [31, 128]
    wv_pack = np.ascontiguousarray(Wv[0][:, None], f32)
    bias_c = np.ascontiguousarray(
        (bq + bm + Wloc @ conv_b)[:, None], f32)
    ident = np.eye(128, dtype=f32)
    ones1 = np.ones((1, 1), dtype=f32)
    return dict(wm_pack=wm_pack, wq_pack=wq_pack, cw_pack=cw_pack,
                wv_pack=wv_pack, bias_c=bias_c, ident=ident, ones1=ones1)


def kernel(query, memory, attention_weights_cum, mask,
           Wq, bq, Wm, bm, Wv, bv, conv_w, conv_b, Wloc):
    from concourse.bass_utils import run_bass_kernel_spmd

    nc = get_program()
    shared = _host_pack(np.asarray(Wq, np.float32), np.asarray(bq, np.float32),
                        np.asarray(Wm, np.float32), np.asarray(bm, np.float32),
                        np.asarray(Wv, np.float32), np.asarray(bv, np.float32),
                        np.asarray(conv_w, np.float32),
                        np.asarray(conv_b, np.float32),
                        np.asarray(Wloc, np.float32))

    query = np.asarray(query, np.float32)
    memory = np.asarray(memory, np.float32)
    awc = np.asarray(attention_weights_cum, np.float32).reshape(B, T)
    maskneg = np.asarray(mask).astype(np.float32) * np.float32(-1e30)

    in_maps = []
    for i in range(N_CORES):
        s = slice(i * BL, (i + 1) * BL)
        in_maps.append({
            "query": np.ascontiguousarray(
                query[s].T.reshape(NQC, 128, BL).transpose(1, 0, 2)
                .reshape(128, NQC * BL)),
            "memory": np.ascontiguousarray(memory[s]),
            "awc": np.ascontiguousarray(awc[s]),
            "maskneg": np.ascontiguousarray(maskneg[s]),
            **shared,
        })

    res = run_bass_kernel_spmd(nc, in_maps, core_ids=list(range(N_CORES)))
    ctx_full = np.concatenate([r["out_ctx"] for r in res.results], axis=0)
    w_full = np.concatenate([r["out_w"] for r in res.results], axis=0)
    return ctx_full, w_full
